# revision 7
# baseline (speedup 1.0000x reference)
"""Two-layer GCN feature extractor on 8 Trainium2 NeuronCores.

v2 pipeline (all shapes hardcoded for N=50000, F=128, E=1.6M + self loops):
  * Nodes sharded across 8 cores (6250 rows each).  Per layer, per core:
      1. transform: h = dinv * (x @ W) via PE matmuls on a host-marshalled
         xT layout (no on-chip transposes); blocks DMA to DRAM h_stage.
      2. AllGather h_stage -> replicated h_full [N, F] bf16 in DRAM.
      3. three gather streams feed one-hot segment-sum matmuls:
           self: sources in own shard, gathered from LOCAL h_stage —
                 runs while the AllGather is still in flight (incl. the
                 GCN self-loop edges),
           lo:   sources < 32768 from h_full,
           hi:   sources >= 32768 from h_full (int16 index limit).
      4. layer 1 accumulates TRANSPOSED (py[f, d], lhsT=slab, rhs=sel) so
         its output y1T [f, d] feeds layer-2 matmuls with no transposes.
      5. finalize: DVE adds self-partial + scales, Scalar engine applies
         relu+bias; layer-2 transform is interleaved into layer-1's
         finalize loop so AllGather #2 launches right at phase end.
  * Host preprocessing: partition edges by destination core, sort by
    (dst block, stream, src), pad each (block, stream) run to 128.
"""

import math
import os
from contextlib import ExitStack

import numpy as np

os.environ.setdefault("MYCRO_LOCAL_CACHE", "1")

# ----------------------------------------------------------------------------
# configuration
# ----------------------------------------------------------------------------


def make_cfg(
    N=50000,
    F=128,
    ncores=8,
    split=32768,
    gchunks=8,
    nqueues=4,
    selb=32,
    lobufs=6,
    hibufs=4,
    selfbufs=3,
    selbufs=2,
):
    assert N % ncores == 0
    rows = N // ncores
    nblk = math.ceil(rows / 128)
    return dict(
        N=N,
        F=F,
        ncores=ncores,
        split=split,
        rows=rows,
        nblk=nblk,
        last_rows=rows - (nblk - 1) * 128,
        gchunks=gchunks,
        nqueues=nqueues,
        selb=selb,
        lobufs=lobufs,
        hibufs=hibufs,
        selfbufs=selfbufs,
        selbufs=selbufs,
    )


FULL_CFG = make_cfg()


# ----------------------------------------------------------------------------
# host-side graph preprocessing
# ----------------------------------------------------------------------------


def preprocess(edge_index, cfg):
    """Partition edges by destination core; per dst block sort into three
    runs (self-shard srcs incl. self-loops, other-lo, other-hi), pad each
    run to a multiple of 128 (chunk counts maxed over cores so the SPMD
    program is uniform), and lay out idx / dst-local streams.

    Returns (sched, per_core, deg) with sched = (c_self, c_lo, c_hi).
    """
    N, ncores, rows, nblk, split = (
        cfg["N"],
        cfg["ncores"],
        cfg["rows"],
        cfg["nblk"],
        cfg["split"],
    )

    src = np.asarray(edge_index[0], dtype=np.int64)
    dst = np.asarray(edge_index[1], dtype=np.int64)
    loops = np.arange(N, dtype=np.int64)
    src = np.concatenate([src, loops])
    dst = np.concatenate([dst, loops])

    deg = np.bincount(dst, minlength=N).astype(np.float32)  # >= 1 (self loops)

    core_of = dst // rows
    per_core_raw = []
    counts = np.zeros((ncores, nblk, 3), dtype=np.int64)
    for k in range(ncores):
        m = core_of == k
        s_k = src[m]
        d_k = dst[m] - k * rows
        blk = d_k >> 7
        dl = (d_k & 127).astype(np.float32)
        own_lo, own_hi = k * rows, (k + 1) * rows
        is_self = (s_k >= own_lo) & (s_k < own_hi)
        # stream id: 0=self, 1=lo(non-self), 2=hi(non-self)
        stream = np.where(is_self, 0, np.where(s_k < split, 1, 2))
        # self stream uses LOCAL indices into h_stage
        s_rel = np.where(is_self, s_k - own_lo, np.where(s_k < split, s_k, s_k - split))
        order = np.lexsort((s_k, stream, blk))
        s_rel, dl, blk, stream = s_rel[order], dl[order], blk[order], stream[order]
        c = np.bincount(blk * 3 + stream, minlength=nblk * 3).reshape(nblk, 3)
        counts[k] = c
        per_core_raw.append((s_rel, dl, c))

    cdiv = lambda a, b: -(-a // b)
    csched = []
    for s in range(3):
        csched.append(
            tuple(
                int(max(cdiv(counts[k, b, s], 128) for k in range(ncores)))
                for b in range(nblk)
            )
        )
    c_self, c_lo, c_hi = csched
    S = [sum(cs) * 128 for cs in csched]

    per_core = []
    for k in range(ncores):
        s_rel, dl_k, c = per_core_raw[k]
        starts = np.concatenate([[0], np.cumsum(c.reshape(-1))])
        idx = [np.zeros(S[s], np.int16) for s in range(3)]
        dst_l = [np.full(S[s], -1.0, np.float32) for s in range(3)]
        pos = [0, 0, 0]
        for b in range(nblk):
            for s in range(3):
                n0 = int(c[b, s])
                o0 = int(starts[b * 3 + s])
                idx[s][pos[s] : pos[s] + n0] = s_rel[o0 : o0 + n0].astype(np.int16)
                dst_l[s][pos[s] : pos[s] + n0] = dl_k[o0 : o0 + n0]
                pos[s] += csched[s][b] * 128

        def arrange_idx(a):  # logical i -> sbuf[i % 16, i // 16], tiled to 128
            if a.size == 0:
                return np.zeros((128, 0), np.int16)
            return np.tile(np.ascontiguousarray(a.reshape(-1, 16).T), (8, 1))

        def arrange_dl(a):  # logical i -> sbuf[i % 128, i // 128]
            if a.size == 0:
                return np.zeros((128, 0), np.float32)
            return np.ascontiguousarray(a.reshape(-1, 128).T)

        per_core.append(
            dict(
                idx_self=arrange_idx(idx[0]),
                idx_lo=arrange_idx(idx[1]),
                idx_hi=arrange_idx(idx[2]),
                dl_self=arrange_dl(dst_l[0]),
                dl_lo=arrange_dl(dst_l[1]),
                dl_hi=arrange_dl(dst_l[2]),
            )
        )

    return (c_self, c_lo, c_hi), per_core, deg


# ----------------------------------------------------------------------------
# bass program
# ----------------------------------------------------------------------------

_PROGRAM_CACHE = {}


def build_program(cfg, sched):
    import concourse.bacc as bacc
    import concourse.mybir as mybir
    import concourse.tile as tile

    c_self, c_lo, c_hi = sched
    N, F, ncores, split = cfg["N"], cfg["F"], cfg["ncores"], cfg["split"]
    rows, nblk, last_rows = cfg["rows"], cfg["nblk"], cfg["last_rows"]
    gchunks = cfg["gchunks"]
    S_self, S_lo, S_hi = sum(c_self) * 128, sum(c_lo) * 128, sum(c_hi) * 128

    f32 = mybir.dt.float32
    bf16 = mybir.dt.bfloat16
    i16 = mybir.dt.int16
    nq = cfg["nqueues"]
    selb = cfg["selb"]
    mul = mybir.AluOpType.mult
    add = mybir.AluOpType.add
    eq = mybir.AluOpType.is_equal
    bypass = mybir.AluOpType.bypass
    relu = mybir.ActivationFunctionType.Relu

    nc = bacc.Bacc(
        "TRN2",
        target_bir_lowering=False,
        debug=False,
        enable_asserts=False,
        num_devices=ncores,
        num_swdge_queues=nq,
    )

    xTd = nc.dram_tensor("xT", [F, nblk * 128], bf16, kind="ExternalInput")
    W1d = nc.dram_tensor("W1", [F, F], bf16, kind="ExternalInput")
    W2d = nc.dram_tensor("W2", [F, F], bf16, kind="ExternalInput")
    b1cd = nc.dram_tensor("b1_col", [128, 1], f32, kind="ExternalInput")
    b2d = nc.dram_tensor("b2_bc", [128, F], f32, kind="ExternalInput")
    dinvd = nc.dram_tensor("dinv", [128, nblk], f32, kind="ExternalInput")
    dinvbcd = nc.dram_tensor("dinv_bc", [128, nblk * 128], f32, kind="ExternalInput")
    iotad = nc.dram_tensor("iota", [128, 128], bf16, kind="ExternalInput")
    stream_dram = {}
    for name, S in (("self", S_self), ("lo", S_lo), ("hi", S_hi)):
        assert S > 0
        stream_dram[name] = (
            nc.dram_tensor(f"idx_{name}", [128, S // 16], i16, kind="ExternalInput"),
            nc.dram_tensor(f"dl_{name}", [128, S // 128], bf16, kind="ExternalInput"),
        )
    yout = nc.dram_tensor("y_out", [rows, F], f32, kind="ExternalOutput")

    h_stage = [nc.dram_tensor(f"h_stage{i}", [rows, F], bf16) for i in (1, 2)]
    h_full = [
        nc.dram_tensor(f"h_full{i}", [N, F], bf16, addr_space="Shared") for i in (1, 2)
    ]

    with tile.TileContext(nc) as tc, ExitStack() as ctx:
        const = ctx.enter_context(tc.tile_pool(name="const", bufs=1))
        xf = ctx.enter_context(tc.tile_pool(name="xf", bufs=3))
        hsf = ctx.enter_context(tc.tile_pool(name="hsf", bufs=3))
        xfp = ctx.enter_context(tc.tile_pool(name="xfp", bufs=2, space="PSUM"))
        sps = ctx.enter_context(tc.tile_pool(name="sps", bufs=2, space="PSUM"))
        gps = ctx.enter_context(tc.tile_pool(name="gps", bufs=2, space="PSUM"))
        gself = ctx.enter_context(tc.tile_pool(name="gself", bufs=cfg["selfbufs"]))
        glo = ctx.enter_context(tc.tile_pool(name="glo", bufs=cfg["lobufs"]))
        ghi = ctx.enter_context(tc.tile_pool(name="ghi", bufs=cfg["hibufs"]))
        selp = ctx.enter_context(tc.tile_pool(name="selp", bufs=cfg["selbufs"]))
        finp = ctx.enter_context(tc.tile_pool(name="finp", bufs=3))
        outp = ctx.enter_context(tc.tile_pool(name="outp", bufs=3))

        # ---- constants ---------------------------------------------------
        def load_const(dram, shape, dtype):
            t = const.tile(shape, dtype, tag=f"c_{dram.name}")
            nc.sync.dma_start(t[:], dram[:])
            return t

        W1s = load_const(W1d, [F, F], bf16)
        W2s = load_const(W2d, [F, F], bf16)
        b1c = load_const(b1cd, [128, 1], f32)
        b2s = load_const(b2d, [128, F], f32)
        iota = load_const(iotad, [128, 128], bf16)
        dinv = load_const(dinvd, [128, nblk], f32)
        dinvbc = load_const(dinvbcd, [128, nblk * 128], f32)
        streams_sb = {}
        for name, S in (("self", S_self), ("lo", S_lo), ("hi", S_hi)):
            ixd, dld = stream_dram[name]
            streams_sb[name] = (
                load_const(ixd, [128, S // 16], i16),
                load_const(dld, [128, S // 128], bf16),
            )

        y1T = const.tile([128, nblk * 128], bf16)  # layer-1 out, [f, d]
        yself1 = const.tile([128, nblk * 128], f32)  # self partial, [f, d]
        yself2 = const.tile([128, nblk * 128], f32)  # self partial, [d, f]

        # ---- gather stream machinery ------------------------------------
        qctr = [0]

        class Stream:
            def __init__(self, name, view, total_chunks, pool):
                self.idx, self.dl = streams_sb[name]
                self.view, self.pool = view, pool
                self.total = total_chunks
                self.pos = 0
                self.slab = None
                self.base = 0
                self.n = 0

            def chunk(self):
                if self.slab is None or self.pos >= self.base + self.n:
                    self.base = self.pos
                    self.n = min(gchunks, self.total - self.pos)
                    nidx = self.n * 128
                    self.slab = self.pool.tile([128, self.n, F], bf16, tag="slab")
                    nc.gpsimd.dma_gather(
                        self.slab[:],
                        self.view,
                        self.idx[:, self.pos * 8 : self.pos * 8 + nidx // 16],
                        nidx,
                        nidx,
                        F,
                        queue_num=qctr[0] % nq,
                        single_packet=True,
                    )
                    qctr[0] += 1
                col = self.pos - self.base
                self.pos += 1
                return self.slab, col

        def run_chunks(st, sched_b, b, py, i, nch, transposed):
            """Emit sel builds + matmuls for block b of stream st."""
            done = 0
            while done < sched_b[b]:
                g = min(selb, sched_b[b] - done)
                p0 = st.pos
                sel = selp.tile([128, selb, 128], bf16, tag="sel")
                nc.vector.tensor_tensor(
                    out=sel[:, :g, :],
                    in0=st.dl[:, p0 : p0 + g].to_broadcast([128, g, 128]),
                    in1=iota[:, None, :].to_broadcast([128, g, 128]),
                    op=eq,
                )
                for j in range(g):
                    slab, col = st.chunk()
                    if transposed:
                        lhsT, rhs = slab[:, col, :], sel[:, j, :]
                    else:
                        lhsT, rhs = sel[:, j, :], slab[:, col, :]
                    nc.tensor.matmul(
                        py[:],
                        lhsT=lhsT,
                        rhs=rhs,
                        start=(i == 0),
                        stop=(i == nch - 1),
                    )
                    i += 1
                done += g
            return i

        # =================================================================
        # layer 1
        # =================================================================
        # ---- transform 1: h_stage1 = dinv * (x @ W1) --------------------
        for b in range(nblk):
            xt = xf.tile([128, F], bf16)
            nc.sync.dma_start(xt[:], xTd[:, b * 128 : (b + 1) * 128])
            ph = xfp.tile([128, F], f32)
            nc.tensor.matmul(ph[:], lhsT=xt[:], rhs=W1s[:], start=True, stop=True)
            hs = hsf.tile([128, F], bf16)
            nc.vector.tensor_scalar(
                out=hs[:], in0=ph[:], scalar1=dinv[:, b : b + 1], scalar2=None, op0=mul
            )
            r = 128 if b < nblk - 1 else last_rows
            nc.sync.dma_start(h_stage[0][b * 128 : b * 128 + r, :], hs[:r, :])

        nc.gpsimd.collective_compute(
            "AllGather",
            bypass,
            replica_groups=[list(range(ncores))],
            ins=[h_stage[0][:]],
            outs=[h_full[0][:]],
        )

        # ---- self pass (overlaps AllGather 1) ---------------------------
        st_self = Stream("self", h_stage[0][:], S_self // 128, gself)
        for b in range(nblk):
            ps = sps.tile([128, F], f32)
            run_chunks(st_self, c_self, b, ps, 0, c_self[b], transposed=True)
            nc.vector.tensor_copy(yself1[:, b * 128 : (b + 1) * 128], ps[:])

        # ---- main pass + fin1 + transform 2 -----------------------------
        st_lo = Stream("lo", h_full[0][:], S_lo // 128, glo)
        st_hi = Stream("hi", h_full[0][split:, :], S_hi // 128, ghi)
        for b in range(nblk):
            py = gps.tile([128, F], f32)
            nch = c_lo[b] + c_hi[b]
            i = run_chunks(st_lo, c_lo, b, py, 0, nch, transposed=True)
            run_chunks(st_hi, c_hi, b, py, i, nch, transposed=True)
            cols = slice(b * 128, (b + 1) * 128)
            # fin1: y1T[:, b] = relu((py + yself1) * dinv_bc + b1)
            t1 = finp.tile([128, F], f32)
            nc.vector.scalar_tensor_tensor(
                out=t1[:], in0=py[:], scalar=1.0, in1=yself1[:, cols],
                op0=bypass, op1=add,
            )
            t2 = finp.tile([128, F], f32)
            nc.vector.tensor_tensor(
                out=t2[:], in0=t1[:], in1=dinvbc[:, cols], op=mul
            )
            nc.scalar.activation(y1T[:, cols], t2[:], relu, bias=b1c[:, 0:1])
            # transform 2 for this block
            ph2 = xfp.tile([128, F], f32)
            nc.tensor.matmul(
                ph2[:], lhsT=y1T[:, cols], rhs=W2s[:], start=True, stop=True
            )
            hs2 = hsf.tile([128, F], bf16)
            nc.vector.tensor_scalar(
                out=hs2[:], in0=ph2[:], scalar1=dinv[:, b : b + 1], scalar2=None,
                op0=mul,
            )
            r = 128 if b < nblk - 1 else last_rows
            nc.sync.dma_start(h_stage[1][b * 128 : b * 128 + r, :], hs2[:r, :])

        nc.gpsimd.collective_compute(
            "AllGather",
            bypass,
            replica_groups=[list(range(ncores))],
            ins=[h_stage[1][:]],
            outs=[h_full[1][:]],
        )

        # =================================================================
        # layer 2
        # =================================================================
        st_self2 = Stream("self", h_stage[1][:], S_self // 128, gself)
        for b in range(nblk):
            ps = sps.tile([128, F], f32)
            run_chunks(st_self2, c_self, b, ps, 0, c_self[b], transposed=False)
            nc.vector.tensor_copy(yself2[:, b * 128 : (b + 1) * 128], ps[:])

        st_lo2 = Stream("lo", h_full[1][:], S_lo // 128, glo)
        st_hi2 = Stream("hi", h_full[1][split:, :], S_hi // 128, ghi)
        for b in range(nblk):
            py = gps.tile([128, F], f32)
            nch = c_lo[b] + c_hi[b]
            i = run_chunks(st_lo2, c_lo, b, py, 0, nch, transposed=False)
            run_chunks(st_hi2, c_hi, b, py, i, nch, transposed=False)
            cols = slice(b * 128, (b + 1) * 128)
            # fin2: out = (py + yself2) * dinv + b2
            t1 = finp.tile([128, F], f32)
            nc.vector.scalar_tensor_tensor(
                out=t1[:], in0=py[:], scalar=1.0, in1=yself2[:, cols],
                op0=bypass, op1=add,
            )
            yt = outp.tile([128, F], f32)
            nc.vector.scalar_tensor_tensor(
                out=yt[:], in0=t1[:], scalar=dinv[:, b : b + 1], in1=b2s[:],
                op0=mul, op1=add,
            )
            r = 128 if b < nblk - 1 else last_rows
            nc.sync.dma_start(yout[b * 128 : b * 128 + r, :], yt[:r, :])

    nc.compile()
    return nc


def get_program(cfg, sched):
    key = (tuple(sorted(cfg.items())), sched)
    if key not in _PROGRAM_CACHE:
        _PROGRAM_CACHE[key] = build_program(cfg, sched)
    return _PROGRAM_CACHE[key]


# ----------------------------------------------------------------------------
# input marshalling + entry point
# ----------------------------------------------------------------------------


def make_in_maps(x, W1, b1, W2, b2, cfg, per_core, deg):
    N, F, ncores, rows, nblk = (
        cfg["N"],
        cfg["F"],
        cfg["ncores"],
        cfg["rows"],
        cfg["nblk"],
    )
    import ml_dtypes

    bf16 = ml_dtypes.bfloat16
    x = np.asarray(x, np.float32)
    W1 = np.ascontiguousarray(np.asarray(W1, np.float32)).astype(bf16)
    W2 = np.ascontiguousarray(np.asarray(W2, np.float32)).astype(bf16)
    b1_col = np.ascontiguousarray(np.asarray(b1, np.float32).reshape(128, 1))
    b2_bc = np.ascontiguousarray(np.broadcast_to(np.asarray(b2, np.float32), (128, F)))
    iota = np.ascontiguousarray(
        np.broadcast_to(np.arange(128, dtype=np.float32), (128, 128))
    ).astype(bf16)
    dinv_full = np.where(deg > 0, 1.0 / np.sqrt(deg), 0.0).astype(np.float32)

    in_maps = []
    for k in range(ncores):
        xk = np.zeros((nblk * 128, F), np.float32)
        xk[:rows] = x[k * rows : (k + 1) * rows]
        xT = np.ascontiguousarray(xk.T).astype(bf16)
        dk = np.ones(nblk * 128, np.float32)
        dk[:rows] = dinv_full[k * rows : (k + 1) * rows]
        dinv = np.ascontiguousarray(dk.reshape(nblk, 128).T)
        dinv_bc = np.ascontiguousarray(np.broadcast_to(dk, (128, nblk * 128)))
        pc = per_core[k]
        in_maps.append(
            dict(
                xT=xT,
                W1=W1,
                W2=W2,
                b1_col=b1_col,
                b2_bc=b2_bc,
                dinv=dinv,
                dinv_bc=dinv_bc,
                iota=iota,
                idx_self=pc["idx_self"],
                idx_lo=pc["idx_lo"],
                idx_hi=pc["idx_hi"],
                dl_self=pc["dl_self"].astype(bf16),
                dl_lo=pc["dl_lo"].astype(bf16),
                dl_hi=pc["dl_hi"].astype(bf16),
            )
        )
    return in_maps


def _ensure_ntff_hook():
    """Register the NTFF profiling hook (missing antenv.axon_hooks shim)."""
    try:
        from antenv.axon_hooks import get_axon_ntff_profile_hook  # noqa: F401

        return True
    except ImportError:
        pass
    try:
        import sys
        import types

        import antenv
        from trn_agent_boot.trn_boot import _ntff_profile_via_ctypes

        hook = _ntff_profile_via_ctypes("/opt/axon/libaxon_pjrt.so")
        if hook is None:
            return False
        mod = types.ModuleType("antenv.axon_hooks")
        mod._hook = hook
        mod.get_axon_ntff_profile_hook = lambda: mod._hook
        mod.set_axon_ntff_profile_hook = lambda h: setattr(mod, "_hook", h)
        sys.modules["antenv.axon_hooks"] = mod
        antenv.axon_hooks = mod
        import concourse.bass_utils as bu

        bu.upload_artifacts = lambda tmpdir: f"local:{tmpdir}"
        return True
    except Exception:
        return False


def run(x, edge_index, W1, b1, W2, b2, cfg, trace=False):
    from concourse.bass_utils import run_bass_kernel_spmd

    if trace:
        trace = _ensure_ntff_hook()

    sched, per_core, deg = preprocess(edge_index, cfg)
    nc = get_program(cfg, sched)
    in_maps = make_in_maps(x, W1, b1, W2, b2, cfg, per_core, deg)
    res = run_bass_kernel_spmd(
        nc, in_maps, list(range(cfg["ncores"])), trace=trace
    )
    out = np.concatenate(
        [res.results[k]["y_out"] for k in range(cfg["ncores"])], axis=0
    )
    return out.astype(np.float32), res


def kernel(x, edge_index, W1, b1, W2, b2):
    out, _ = run(x, edge_index, W1, b1, W2, b2, FULL_CFG)
    return out


# revision 55
# speedup vs baseline: 1.9558x; 1.9558x over previous
"""Two-layer GCN feature extractor on 8 Trainium2 NeuronCores.

v2 pipeline (all shapes hardcoded for N=50000, F=128, E=1.6M + self loops):
  * Nodes sharded across 8 cores (6250 rows each).  Per layer, per core:
      1. transform: h = dinv * (x @ W) via PE matmuls on a host-marshalled
         xT layout (no on-chip transposes); blocks DMA to DRAM h_stage.
      2. AllGather h_stage -> replicated h_full [N, F] bf16 in DRAM.
      3. three gather streams feed one-hot segment-sum matmuls:
           self: sources in own shard, gathered from LOCAL h_stage —
                 runs while the AllGather is still in flight (incl. the
                 GCN self-loop edges),
           lo:   sources < 32768 from h_full,
           hi:   sources >= 32768 from h_full (int16 index limit).
      4. layer 1 accumulates TRANSPOSED (py[f, d], lhsT=slab, rhs=sel) so
         its output y1T [f, d] feeds layer-2 matmuls with no transposes.
      5. finalize: DVE adds self-partial + scales, Scalar engine applies
         relu+bias; layer-2 transform is interleaved into layer-1's
         finalize loop so AllGather #2 launches right at phase end.
  * Host preprocessing: partition edges by destination core, sort by
    (dst block, stream, src), pad each (block, stream) run to 128.
"""

import math
import os
from contextlib import ExitStack

import numpy as np

os.environ.setdefault("MYCRO_LOCAL_CACHE", "1")

# ----------------------------------------------------------------------------
# configuration
# ----------------------------------------------------------------------------


def make_cfg(
    N=50000,
    F=128,
    ncores=8,
    split=32768,
    gchunks=8,
    nqueues=4,
    selb=32,
    lobufs=6,
    hibufs=4,
    selfbufs=3,
    selbufs=3,
    t1x=True,
    selfstream=False,
    inter_t2=True,
    ag_chunks=5,
):
    assert N % ncores == 0
    rows = N // ncores
    nblk = math.ceil(rows / 128)
    return dict(
        N=N,
        F=F,
        ncores=ncores,
        split=split,
        rows=rows,
        nblk=nblk,
        last_rows=rows - (nblk - 1) * 128,
        gchunks=gchunks,
        nqueues=nqueues,
        selb=selb,
        lobufs=lobufs,
        hibufs=hibufs,
        selfbufs=selfbufs,
        selbufs=selbufs,
        t1x=t1x,
        selfstream=selfstream,
        inter_t2=inter_t2,
        ag_chunks=ag_chunks,
    )


FULL_CFG = make_cfg()


# ----------------------------------------------------------------------------
# host-side graph preprocessing
# ----------------------------------------------------------------------------


def preprocess(edge_index, cfg):
    """Partition edges by destination core; per dst block sort into three
    runs (self-shard srcs incl. self-loops, other-lo, other-hi), pad each
    run to a multiple of 128 (chunk counts maxed over cores so the SPMD
    program is uniform), and lay out idx / dst-local streams.

    Returns (sched, per_core, deg) with sched = (c_self, c_lo, c_hi).
    """
    N, ncores, rows, nblk, split = (
        cfg["N"],
        cfg["ncores"],
        cfg["rows"],
        cfg["nblk"],
        cfg["split"],
    )

    src = np.asarray(edge_index[0], dtype=np.int64)
    dst = np.asarray(edge_index[1], dtype=np.int64)
    loops = np.arange(N, dtype=np.int64)
    src = np.concatenate([src, loops])
    dst = np.concatenate([dst, loops])

    deg = np.bincount(dst, minlength=N).astype(np.float32)  # >= 1 (self loops)

    # h_full is chunk-major ([agch][ncores][ag_rows]) so chunked AllGathers
    # write contiguous slices; remap source node ids into that layout.
    agch = cfg.get("ag_chunks", 1)
    assert rows % agch == 0
    assert agch == 1 or not cfg.get("selfstream", False), (
        "selfstream needs original-order indices"
    )
    ag_rows = rows // agch
    k_src = src // rows
    r_src = src % rows
    src = (r_src // ag_rows) * (ncores * ag_rows) + k_src * ag_rows + (r_src % ag_rows)

    core_of = dst // rows
    per_core_raw = []
    counts = np.zeros((ncores, nblk, 3), dtype=np.int64)
    for k in range(ncores):
        m = core_of == k
        s_k = src[m]
        d_k = dst[m] - k * rows
        blk = d_k >> 7
        dl = (d_k & 127).astype(np.float32)
        own_lo, own_hi = k * rows, (k + 1) * rows
        is_self = (s_k >= own_lo) & (s_k < own_hi)
        if not cfg.get("selfstream", True):
            is_self = np.zeros_like(is_self)
        # stream id: 0=self, 1=lo(non-self), 2=hi(non-self)
        stream = np.where(is_self, 0, np.where(s_k < split, 1, 2))
        # self stream uses LOCAL indices into h_stage
        s_rel = np.where(is_self, s_k - own_lo, np.where(s_k < split, s_k, s_k - split))
        order = np.lexsort((s_k, stream, blk))
        s_rel, dl, blk, stream = s_rel[order], dl[order], blk[order], stream[order]
        c = np.bincount(blk * 3 + stream, minlength=nblk * 3).reshape(nblk, 3)
        counts[k] = c
        per_core_raw.append((s_rel, dl, c))

    cdiv = lambda a, b: -(-a // b)
    csched = []
    for s in range(3):
        csched.append(
            tuple(
                int(max(cdiv(counts[k, b, s], 128) for k in range(ncores)))
                for b in range(nblk)
            )
        )
    c_self, c_lo, c_hi = csched
    S = [sum(cs) * 128 for cs in csched]

    per_core = []
    for k in range(ncores):
        s_rel, dl_k, c = per_core_raw[k]
        starts = np.concatenate([[0], np.cumsum(c.reshape(-1))])
        idx = [np.zeros(S[s], np.int16) for s in range(3)]
        dst_l = [np.full(S[s], -1.0, np.float32) for s in range(3)]
        pos = [0, 0, 0]
        for b in range(nblk):
            for s in range(3):
                n0 = int(c[b, s])
                o0 = int(starts[b * 3 + s])
                idx[s][pos[s] : pos[s] + n0] = s_rel[o0 : o0 + n0].astype(np.int16)
                dst_l[s][pos[s] : pos[s] + n0] = dl_k[o0 : o0 + n0]
                pos[s] += csched[s][b] * 128

        def arrange_idx(a):  # logical i -> sbuf[i % 16, i // 16], tiled to 128
            if a.size == 0:
                return np.zeros((128, 0), np.int16)
            return np.tile(np.ascontiguousarray(a.reshape(-1, 16).T), (8, 1))

        def arrange_dl(a):  # logical i -> sbuf[i % 128, i // 128]
            if a.size == 0:
                return np.zeros((128, 0), np.float32)
            return np.ascontiguousarray(a.reshape(-1, 128).T)

        per_core.append(
            dict(
                idx_self=arrange_idx(idx[0]),
                idx_lo=arrange_idx(idx[1]),
                idx_hi=arrange_idx(idx[2]),
                dl_self=arrange_dl(dst_l[0]),
                dl_lo=arrange_dl(dst_l[1]),
                dl_hi=arrange_dl(dst_l[2]),
                raw_idx=idx,  # unpadded-layout (padded values) per stream
            )
        )

    # Per-chunk max table row touched (over all cores) so gather calls can
    # use narrow row-range views of h_full (start before all AG chunks land).
    chunk_max = []
    for s in range(3):
        nchunks = S[s] // 128
        cm = np.zeros(nchunks, np.int64)
        for k in range(ncores):
            a = per_core[k]["raw_idx"][s].astype(np.int64)
            if a.size:
                m = a.reshape(nchunks, 128).max(axis=1)
                cm = np.maximum(cm, m)
        chunk_max.append(tuple(int(v) for v in cm))
    for pc in per_core:
        del pc["raw_idx"]

    return (c_self, c_lo, c_hi, tuple(chunk_max)), per_core, deg


# ----------------------------------------------------------------------------
# bass program
# ----------------------------------------------------------------------------

_PROGRAM_CACHE = {}


def build_program(cfg, sched):
    import concourse.bacc as bacc
    import concourse.mybir as mybir
    import concourse.tile as tile

    c_self, c_lo, c_hi, chunk_max = sched
    N, F, ncores, split = cfg["N"], cfg["F"], cfg["ncores"], cfg["split"]
    rows, nblk, last_rows = cfg["rows"], cfg["nblk"], cfg["last_rows"]
    gchunks = cfg["gchunks"]
    S_self, S_lo, S_hi = sum(c_self) * 128, sum(c_lo) * 128, sum(c_hi) * 128

    f32 = mybir.dt.float32
    bf16 = mybir.dt.bfloat16
    i16 = mybir.dt.int16
    nq = cfg["nqueues"]
    selb = cfg["selb"]
    mul = mybir.AluOpType.mult
    add = mybir.AluOpType.add
    eq = mybir.AluOpType.is_equal
    bypass = mybir.AluOpType.bypass
    relu = mybir.ActivationFunctionType.Relu

    nc = bacc.Bacc(
        "TRN2",
        target_bir_lowering=False,
        debug=False,
        enable_asserts=False,
        num_devices=ncores,
        num_swdge_queues=nq,
    )

    t1x = cfg["t1x"]
    xTd = nc.dram_tensor("xT", [F, nblk * 128], bf16, kind="ExternalInput")
    W1d = nc.dram_tensor("W1", [F, F], bf16, kind="ExternalInput")
    W2d = nc.dram_tensor("W2", [F, F], bf16, kind="ExternalInput")
    b1cd = nc.dram_tensor("b1_col", [128, 1], f32, kind="ExternalInput")
    b1bd = nc.dram_tensor("b1_bc", [128, F], f32, kind="ExternalInput")
    b2d = nc.dram_tensor("b2_bc", [128, F], f32, kind="ExternalInput")
    dinvd = nc.dram_tensor("dinv", [128, nblk], f32, kind="ExternalInput")
    dinvbcd = nc.dram_tensor("dinv_bc", [128, nblk * 128], f32, kind="ExternalInput")
    iotad = nc.dram_tensor("iota", [128, 128], bf16, kind="ExternalInput")
    stream_dram = {}
    for name, S in (("self", S_self), ("lo", S_lo), ("hi", S_hi)):
        if S == 0:
            continue
        stream_dram[name] = (
            nc.dram_tensor(f"idx_{name}", [128, S // 16], i16, kind="ExternalInput"),
            nc.dram_tensor(f"dl_{name}", [128, S // 128], bf16, kind="ExternalInput"),
        )
    yout = nc.dram_tensor("y_out", [rows, F], f32, kind="ExternalOutput")

    h_stage = [nc.dram_tensor(f"h_stage{i}", [rows, F], bf16) for i in (1, 2)]
    h_full = [
        nc.dram_tensor(f"h_full{i}", [N, F], bf16, addr_space="Shared") for i in (1, 2)
    ]

    with tile.TileContext(nc) as tc, ExitStack() as ctx:
        const = ctx.enter_context(tc.tile_pool(name="const", bufs=1))
        xf = ctx.enter_context(tc.tile_pool(name="xf", bufs=3))
        hsf = ctx.enter_context(tc.tile_pool(name="hsf", bufs=3))
        xfp = ctx.enter_context(tc.tile_pool(name="xfp", bufs=2, space="PSUM"))
        sps = ctx.enter_context(
            tc.tile_pool(name="sps", bufs=2 if cfg["t1x"] else 1, space="PSUM")
        )
        gps = ctx.enter_context(tc.tile_pool(name="gps", bufs=2, space="PSUM"))
        tpp = (
            None
            if cfg["t1x"]
            else ctx.enter_context(tc.tile_pool(name="tpp", bufs=1, space="PSUM"))
        )

        gself = ctx.enter_context(tc.tile_pool(name="gself", bufs=cfg["selfbufs"]))
        glo = ctx.enter_context(tc.tile_pool(name="glo", bufs=cfg["lobufs"]))
        ghi = ctx.enter_context(tc.tile_pool(name="ghi", bufs=cfg["hibufs"]))
        selp = ctx.enter_context(tc.tile_pool(name="selp", bufs=cfg["selbufs"]))
        finp = ctx.enter_context(tc.tile_pool(name="finp", bufs=3))
        outp = ctx.enter_context(tc.tile_pool(name="outp", bufs=3))

        # ---- constants ---------------------------------------------------
        def load_const(dram, shape, dtype):
            t = const.tile(shape, dtype, tag=f"c_{dram.name}")
            nc.sync.dma_start(t[:], dram[:])
            return t

        W1s = load_const(W1d, [F, F], bf16)
        W2s = load_const(W2d, [F, F], bf16)
        b1c = load_const(b1cd, [128, 1], f32)
        b1b = load_const(b1bd, [128, F], f32)
        b2s = load_const(b2d, [128, F], f32)
        iota = load_const(iotad, [128, 128], bf16)
        dinv = load_const(dinvd, [128, nblk], f32)
        dinvbc = load_const(dinvbcd, [128, nblk * 128], f32) if t1x else None
        if not t1x:
            from concourse.masks import make_identity

            ident = const.tile([128, 128], bf16)
            make_identity(nc, ident[:])
        streams_sb = {}
        for name, S in (("self", S_self), ("lo", S_lo), ("hi", S_hi)):
            if S == 0:
                continue
            ixd, dld = stream_dram[name]
            streams_sb[name] = (
                load_const(ixd, [128, S // 16], i16),
                load_const(dld, [128, S // 128], bf16),
            )

        # layer-1 output: [f, d] when t1x else [d, f]
        y1T = const.tile([128, nblk * 128], bf16)
        yself1 = const.tile([128, nblk * 128], f32)  # self partial, same layout
        yself2 = const.tile([128, nblk * 128], f32)  # self partial, [d, f]

        # ---- gather stream machinery ------------------------------------
        qctr = [0]

        class Stream:
            def __init__(self, name, view_fn, total_chunks, pool, cmax=None):
                self.idx, self.dl = streams_sb[name]
                self.view_fn, self.pool = view_fn, pool
                self.cmax = cmax  # per-chunk max row touched (view narrowing)
                self.total = total_chunks
                self.pos = 0
                self.slab = None
                self.base = 0
                self.n = 0

            def chunk(self):
                if self.slab is None or self.pos >= self.base + self.n:
                    self.base = self.pos
                    self.n = min(gchunks, self.total - self.pos)
                    nidx = self.n * 128
                    if self.cmax is None:
                        view = self.view_fn(None)
                    else:
                        cap = 1 + max(self.cmax[self.pos : self.pos + self.n])
                        view = self.view_fn(cap)
                    self.slab = self.pool.tile([128, self.n, F], bf16, tag="slab")
                    nc.gpsimd.dma_gather(
                        self.slab[:],
                        view,
                        self.idx[:, self.pos * 8 : self.pos * 8 + nidx // 16],
                        nidx,
                        nidx,
                        F,
                        queue_num=qctr[0] % nq,
                        single_packet=True,
                    )
                    qctr[0] += 1
                col = self.pos - self.base
                self.pos += 1
                return self.slab, col

        def run_chunks(st, sched_b, b, py, i, nch, transposed):
            """Emit sel builds + matmuls for block b of stream st."""
            done = 0
            while done < sched_b[b]:
                g = min(selb, sched_b[b] - done)
                p0 = st.pos
                sel = selp.tile([128, selb, 128], bf16, tag="sel")
                nc.vector.tensor_tensor(
                    out=sel[:, :g, :],
                    in0=st.dl[:, p0 : p0 + g].to_broadcast([128, g, 128]),
                    in1=iota[:, None, :].to_broadcast([128, g, 128]),
                    op=eq,
                )
                for j in range(g):
                    slab, col = st.chunk()
                    if transposed:
                        lhsT, rhs = slab[:, col, :], sel[:, j, :]
                    else:
                        lhsT, rhs = sel[:, j, :], slab[:, col, :]
                    nc.tensor.matmul(
                        py[:],
                        lhsT=lhsT,
                        rhs=rhs,
                        start=(i == 0),
                        stop=(i == nch - 1),
                    )
                    i += 1
                done += g
            return i

        # =================================================================
        # layer 1
        # =================================================================
        # chunked AllGather: fire chunk c as soon as its staging rows land
        agch = cfg["ag_chunks"]
        assert rows % agch == 0
        ag_rows = rows // agch
        groups = [list(range(ncores))]

        def ag_chunk(li, c):
            # h_full is laid out chunk-major: [agch][ncores][ag_rows][F], so
            # each chunk's AllGather output slice is contiguous (host remaps
            # the gather indices to match).
            band = ncores * ag_rows
            nc.gpsimd.collective_compute(
                "AllGather",
                bypass,
                replica_groups=groups,
                ins=[h_stage[li][c * ag_rows : (c + 1) * ag_rows, :]],
                outs=[h_full[li][c * band : (c + 1) * band, :]],
            )

        def ag_after_block(li, b):
            """Fire any AG chunks fully staged once block b is written."""
            hi_row = b * 128 + (128 if b < nblk - 1 else last_rows)
            lo_row = b * 128
            for c in range(agch):
                end = (c + 1) * ag_rows
                if lo_row < end <= hi_row or (b == nblk - 1 and end > hi_row):
                    ag_chunk(li, c)

        # ---- transform 1: h_stage1 = dinv * (x @ W1) --------------------
        XB = 8  # blocks per xT load (2KB-per-partition contiguous reads)
        xt8 = None
        for b in range(nblk):
            if b % XB == 0:
                nxb = min(XB, nblk - b)
                xt8 = xf.tile([128, XB * 128], bf16, tag="xt8")
                nc.sync.dma_start(
                    xt8[:, : nxb * 128], xTd[:, b * 128 : (b + nxb) * 128]
                )
            ph = xfp.tile([128, F], f32)
            nc.tensor.matmul(
                ph[:],
                lhsT=xt8[:, (b % XB) * 128 : (b % XB + 1) * 128],
                rhs=W1s[:],
                start=True,
                stop=True,
            )
            hs = hsf.tile([128, F], bf16)
            nc.vector.tensor_scalar(
                out=hs[:], in0=ph[:], scalar1=dinv[:, b : b + 1], scalar2=None, op0=mul
            )
            r = 128 if b < nblk - 1 else last_rows
            nc.scalar.dma_start(h_stage[0][b * 128 : b * 128 + r, :], hs[:r, :])
            ag_after_block(0, b)

        # ---- self pass (overlaps AllGather 1) ---------------------------
        if S_self:
            st_self = Stream("self", lambda cap: h_stage[0][:], S_self // 128, gself)
            for b in range(nblk):
                ps = sps.tile([128, F], f32)
                run_chunks(st_self, c_self, b, ps, 0, c_self[b], transposed=t1x)
                nc.vector.tensor_copy(yself1[:, b * 128 : (b + 1) * 128], ps[:])

        # ---- main pass + fin1 + transform 2 -----------------------------
        st_lo = Stream(
            "lo", lambda cap: h_full[0][:cap, :], S_lo // 128, glo, chunk_max[1]
        )
        st_hi = Stream(
            "hi",
            lambda cap: h_full[0][split : split + cap, :],
            S_hi // 128,
            ghi,
            chunk_max[2],
        )
        def transform2_block(b):
            cols = slice(b * 128, (b + 1) * 128)
            if t1x:
                y1T_blk = y1T[:, cols]
            else:
                pT = tpp.tile([128, F], bf16)
                nc.tensor.transpose(pT[:], y1T[:, cols], ident[:])
                yt_ = finp.tile([128, F], bf16, tag="y1Tb")
                nc.vector.tensor_copy(yt_[:], pT[:])
                y1T_blk = yt_[:]
            ph2 = xfp.tile([128, F], f32)
            nc.tensor.matmul(
                ph2[:], lhsT=y1T_blk, rhs=W2s[:], start=True, stop=True
            )
            hs2 = hsf.tile([128, F], bf16)
            nc.vector.tensor_scalar(
                out=hs2[:], in0=ph2[:], scalar1=dinv[:, b : b + 1], scalar2=None,
                op0=mul,
            )
            r = 128 if b < nblk - 1 else last_rows
            nc.scalar.dma_start(h_stage[1][b * 128 : b * 128 + r, :], hs2[:r, :])
            ag_after_block(1, b)

        for b in range(nblk):
            py = gps.tile([128, F], f32)
            nch = c_lo[b] + c_hi[b]
            i = run_chunks(st_lo, c_lo, b, py, 0, nch, transposed=t1x)
            run_chunks(st_hi, c_hi, b, py, i, nch, transposed=t1x)
            cols = slice(b * 128, (b + 1) * 128)
            if S_self:
                t1 = finp.tile([128, F], f32)
                nc.vector.scalar_tensor_tensor(
                    out=t1[:], in0=py[:], scalar=1.0, in1=yself1[:, cols],
                    op0=bypass, op1=add,
                )
                t1 = t1[:]
            else:
                t1 = py[:]
            if t1x:
                # fin1: y1T[:, b] = relu((py + yself1) * dinv_bc + b1)
                t2 = finp.tile([128, F], f32)
                nc.vector.tensor_tensor(
                    out=t2[:], in0=t1, in1=dinvbc[:, cols], op=mul
                )
                nc.scalar.activation(y1T[:, cols], t2[:], relu, bias=b1c[:, 0:1])
            else:
                # fin1: y1[d, f] = relu(t1 * dinv + b1)
                ys = finp.tile([128, F], f32)
                nc.vector.scalar_tensor_tensor(
                    out=ys[:], in0=t1, scalar=dinv[:, b : b + 1], in1=b1b[:],
                    op0=mul, op1=add,
                )
                nc.vector.tensor_scalar(
                    out=y1T[:, cols], in0=ys[:], scalar1=0.0, scalar2=None,
                    op0=mybir.AluOpType.max,
                )
            if cfg["inter_t2"]:
                transform2_block(b)

        if not cfg["inter_t2"]:
            for b in range(nblk):
                transform2_block(b)

        # =================================================================
        # layer 2
        # =================================================================
        if S_self:
            st_self2 = Stream("self", lambda cap: h_stage[1][:], S_self // 128, gself)
            for b in range(nblk):
                ps = sps.tile([128, F], f32)
                run_chunks(st_self2, c_self, b, ps, 0, c_self[b], transposed=False)
                nc.vector.tensor_copy(yself2[:, b * 128 : (b + 1) * 128], ps[:])

        st_lo2 = Stream(
            "lo", lambda cap: h_full[1][:cap, :], S_lo // 128, glo, chunk_max[1]
        )
        st_hi2 = Stream(
            "hi",
            lambda cap: h_full[1][split : split + cap, :],
            S_hi // 128,
            ghi,
            chunk_max[2],
        )
        for b in range(nblk):
            py = gps.tile([128, F], f32)
            nch = c_lo[b] + c_hi[b]
            i = run_chunks(st_lo2, c_lo, b, py, 0, nch, transposed=False)
            run_chunks(st_hi2, c_hi, b, py, i, nch, transposed=False)
            cols = slice(b * 128, (b + 1) * 128)
            # fin2: out = (py + yself2) * dinv + b2
            if S_self:
                t1 = finp.tile([128, F], f32)
                nc.vector.scalar_tensor_tensor(
                    out=t1[:], in0=py[:], scalar=1.0, in1=yself2[:, cols],
                    op0=bypass, op1=add,
                )
                t1 = t1[:]
            else:
                t1 = py[:]
            yt = outp.tile([128, F], f32)
            nc.vector.scalar_tensor_tensor(
                out=yt[:], in0=t1, scalar=dinv[:, b : b + 1], in1=b2s[:],
                op0=mul, op1=add,
            )
            r = 128 if b < nblk - 1 else last_rows
            nc.sync.dma_start(yout[b * 128 : b * 128 + r, :], yt[:r, :])

    nc.compile()
    return nc


def get_program(cfg, sched):
    key = (tuple(sorted(cfg.items())), sched)
    if key not in _PROGRAM_CACHE:
        _PROGRAM_CACHE[key] = build_program(cfg, sched)
    return _PROGRAM_CACHE[key]


# ----------------------------------------------------------------------------
# input marshalling + entry point
# ----------------------------------------------------------------------------


def make_in_maps(x, W1, b1, W2, b2, cfg, per_core, deg):
    N, F, ncores, rows, nblk = (
        cfg["N"],
        cfg["F"],
        cfg["ncores"],
        cfg["rows"],
        cfg["nblk"],
    )
    import ml_dtypes

    bf16 = ml_dtypes.bfloat16
    x = np.asarray(x, np.float32)
    W1 = np.ascontiguousarray(np.asarray(W1, np.float32)).astype(bf16)
    W2 = np.ascontiguousarray(np.asarray(W2, np.float32)).astype(bf16)
    b1_col = np.ascontiguousarray(np.asarray(b1, np.float32).reshape(128, 1))
    b1_bc = np.ascontiguousarray(np.broadcast_to(np.asarray(b1, np.float32), (128, F)))
    b2_bc = np.ascontiguousarray(np.broadcast_to(np.asarray(b2, np.float32), (128, F)))
    iota = np.ascontiguousarray(
        np.broadcast_to(np.arange(128, dtype=np.float32), (128, 128))
    ).astype(bf16)
    dinv_full = np.where(deg > 0, 1.0 / np.sqrt(deg), 0.0).astype(np.float32)

    in_maps = []
    for k in range(ncores):
        xk = np.zeros((nblk * 128, F), np.float32)
        xk[:rows] = x[k * rows : (k + 1) * rows]
        xT = np.ascontiguousarray(xk.T).astype(bf16)
        dk = np.ones(nblk * 128, np.float32)
        dk[:rows] = dinv_full[k * rows : (k + 1) * rows]
        dinv = np.ascontiguousarray(dk.reshape(nblk, 128).T)
        dinv_bc = np.ascontiguousarray(np.broadcast_to(dk, (128, nblk * 128)))
        pc = per_core[k]
        in_maps.append(
            dict(
                xT=xT,
                W1=W1,
                W2=W2,
                b1_col=b1_col,
                b1_bc=b1_bc,
                b2_bc=b2_bc,
                dinv=dinv,
                dinv_bc=dinv_bc,
                iota=iota,
            )
        )
        for s in ("self", "lo", "hi"):
            if pc[f"idx_{s}"].size:
                in_maps[-1][f"idx_{s}"] = pc[f"idx_{s}"]
                in_maps[-1][f"dl_{s}"] = pc[f"dl_{s}"].astype(bf16)
    return in_maps


def _ensure_ntff_hook():
    """Register the NTFF profiling hook (missing antenv.axon_hooks shim)."""
    try:
        from antenv.axon_hooks import get_axon_ntff_profile_hook  # noqa: F401

        return True
    except ImportError:
        pass
    try:
        import sys
        import types

        import antenv
        from trn_agent_boot.trn_boot import _ntff_profile_via_ctypes

        hook = _ntff_profile_via_ctypes("/opt/axon/libaxon_pjrt.so")
        if hook is None:
            return False
        mod = types.ModuleType("antenv.axon_hooks")
        mod._hook = hook
        mod.get_axon_ntff_profile_hook = lambda: mod._hook
        mod.set_axon_ntff_profile_hook = lambda h: setattr(mod, "_hook", h)
        sys.modules["antenv.axon_hooks"] = mod
        antenv.axon_hooks = mod
        import concourse.bass_utils as bu

        bu.upload_artifacts = lambda tmpdir: f"local:{tmpdir}"
        return True
    except Exception:
        return False


def run(x, edge_index, W1, b1, W2, b2, cfg, trace=False):
    from concourse.bass_utils import run_bass_kernel_spmd

    if trace:
        trace = _ensure_ntff_hook()

    sched, per_core, deg = preprocess(edge_index, cfg)
    nc = get_program(cfg, sched)
    in_maps = make_in_maps(x, W1, b1, W2, b2, cfg, per_core, deg)
    res = run_bass_kernel_spmd(
        nc, in_maps, list(range(cfg["ncores"])), trace=trace
    )
    out = np.concatenate(
        [res.results[k]["y_out"] for k in range(cfg["ncores"])], axis=0
    )
    return out.astype(np.float32), res


def kernel(x, edge_index, W1, b1, W2, b2):
    out, _ = run(x, edge_index, W1, b1, W2, b2, FULL_CFG)
    return out
